# revision 48
# baseline (speedup 1.0000x reference)
"""LocationSensitiveSoftAttention on 8 Trainium2 NeuronCores (Bass/Tile). v2

Contract: kernel(**inputs) takes the FULL unsharded inputs (numpy arrays, keys
as in setup_inputs()) and returns the FULL output [64, 1, 256] fp32.

Strategy: data-parallel over batch B=64 -> 8 batches per core; weights
replicated and folded on host. Math restructure (exact up to fp rounding):
  pre^T[u,t] = (Wm@We)^T mem[b]^T + G^T shifted + r[b]  broadcast over t
     where G = conv_w^T @ (Wl@We)  [31,256] (conv folded into weights),
           r[b] = q1[b] @ (Wq@We) + c0,
           c0 = (bq+bm+bl)@We + be + conv_b@(Wl@We)
  h = tanh(pre^T); energy^T[t] = v_a @ h   (h stationary, fp8 DoubleRow)
  s = sigmoid(energy); w = state + s/sum(s)
  context = (w @ mem) @ Wm + (sum(state)+1) * bm

Key implementation points:
- The attention branch (pre/h/energy) runs fp8 e4m3 + DoubleRow (2 MACs/
  cell): its errors are attenuated ~2000x at the output because alignment
  sums to 1 over T=2048 while state ~ U[0,1) dominates w. The context GEMV
  (w @ mem) stays bf16 for output precision. HBM traffic is 3 bytes/element
  of memory (bf16 natural layout + fp8 transposed) vs 4 for 2x bf16.
- Energies are produced TRANSPOSED ([128t x 16] PSUM tile, h stationary /
  v_a moving, N=1 matmuls) so sigmoid/normalize/cumulate are full-width
  ops - never single-partition rows.
- pre PSUM tiles are drained to SBUF by DVE copies so the PE never stalls
  on tanh (ACT) to reuse its accumulation banks; tanh reads SBUF.
- Per-batch software pipeline: norm(b-1) / attention(b) / context(b-1)
  interleaved in PE issue order; bulk loads via SWDGE (gpsimd) keeping
  the Pool engine free of compute; all shifted-state tiles preloaded so
  they never queue behind bulk loads.
- The last batch takes a transpose-free output path (nat-stationary N=1
  GEMVs giving cv^T directly) to minimize the serial tail; the other 7
  batches' outputs go through a PE-transpose + Wm GEMM per 4-batch group,
  fully overlapped with the pipeline.
"""

import sys

for _p in ("/root/.axon_site", "/root/.axon_site/_ro/trn_rl_repo",
           "/root/.axon_site/_ro/pypackages", "/opt/trn_rl_repo"):
    if _p not in sys.path:
        sys.path.append(_p)

import numpy as np
import ml_dtypes

B, TQ, T = 64, 2, 2048
HID, ENC, U, FILT, K = 1024, 512, 256, 32, 31
N_CORES = 8
PB = B // N_CORES  # batches per core
PAD = K // 2  # 15
NT = T // 128  # 16 t-tiles
SPADW = T + 2 * PAD + 2  # 2080, padded state row length (even)

BF16 = ml_dtypes.bfloat16
FP8 = ml_dtypes.float8_e4m3

_BUILT = {}
TRACE = False
LAST_RESULTS = None


def _build_nc(repeat=1):
    import concourse.bacc as bacc
    import concourse.mybir as mybir
    import concourse.tile as tile
    import concourse.bass as bass

    f32 = mybir.dt.float32
    bf16 = mybir.dt.bfloat16
    fp8 = mybir.dt.float8e4
    AF = mybir.ActivationFunctionType
    ALU = mybir.AluOpType
    AX = mybir.AxisListType
    DR = mybir.MatmulPerfMode.DoubleRow

    nc = bacc.Bacc("TRN2", target_bir_lowering=False, debug=False,
                   num_devices=N_CORES)

    # ---- DRAM I/O ----
    natc_d = nc.dram_tensor("natc", [PB, 128, NT * ENC], bf16, kind="ExternalInput")
    memt8_d = nc.dram_tensor("memt8", [PB, 128, 2, 2, T], fp8, kind="ExternalInput")
    spadb8_d = nc.dram_tensor("spadb8", [PB, SPADW], fp8, kind="ExternalInput")
    statet_d = nc.dram_tensor("statet", [128, PB, NT], bf16, kind="ExternalInput")
    wmwe8_d = nc.dram_tensor("wmwe8", [128, 2, 2, U], fp8, kind="ExternalInput")
    g8_d = nc.dram_tensor("g8", [16, 2, U], fp8, kind="ExternalInput")
    vat8_d = nc.dram_tensor("vat8", [128, 2, 1], fp8, kind="ExternalInput")
    wqwe_d = nc.dram_tensor("wqwe", [128, 8, 2, 128], fp8, kind="ExternalInput")
    q1t_d = nc.dram_tensor("q1t", [128, 8, PB], fp8, kind="ExternalInput")
    c0row_d = nc.dram_tensor("c0row", [1, 2, 128], bf16, kind="ExternalInput")
    wmb_d = nc.dram_tensor("wmb", [128, 4, U], bf16, kind="ExternalInput")
    bmrow_d = nc.dram_tensor("bmrow", [1, U], bf16, kind="ExternalInput")
    sigt_d = nc.dram_tensor("sigt", [1, 2, 128], bf16, kind="ExternalInput")
    idb_d = nc.dram_tensor("idb", [128, 128], bf16, kind="ExternalInput")
    out_d = nc.dram_tensor("out", [PB, U], f32, kind="ExternalOutput")

    with tile.TileContext(nc) as tc:
        with (
            tc.tile_pool(name="consts", bufs=1) as consts,
            tc.tile_pool(name="nat", bufs=5) as natp,
            tc.tile_pool(name="memT", bufs=5) as mtp,
            tc.tile_pool(name="shf", bufs=8) as shfp,
            tc.tile_pool(name="hb", bufs=4) as hbp,
            tc.tile_pool(name="rows", bufs=3) as rowp,
            tc.tile_pool(name="preb", bufs=6) as prebp,
            tc.tile_pool(name="psPre", bufs=2, space="PSUM") as psPre,
            tc.tile_pool(name="psEn", bufs=2, space="PSUM") as psEn,
            tc.tile_pool(name="psCv", bufs=2, space="PSUM") as psCv,
        ):
          def _body():
              # ---- head (batch-0-gating) constants, in priority order ----
              with tc.high_priority():
                  wmwe8_sb = consts.tile([128, 2, 2, U], fp8, tag="wmwe8")
                  nc.sync.dma_start(out=wmwe8_sb[:], in_=wmwe8_d.ap())
                  g8_sb = consts.tile([16, 2, U], fp8, tag="g8")
                  nc.sync.dma_start(out=g8_sb[:], in_=g8_d.ap())
                  vat8_sb = consts.tile([128, 2, 1], fp8, tag="vat8")
                  nc.sync.dma_start(out=vat8_sb[:], in_=vat8_d.ap())
                  q1t_sb = consts.tile([128, 8, PB], fp8, tag="q1t")
                  nc.sync.dma_start(out=q1t_sb[:], in_=q1t_d.ap())
                  c0row_sb = consts.tile([1, 2, 128], bf16, tag="c0row")
                  nc.sync.dma_start(out=c0row_sb[:], in_=c0row_d.ap())
                  memt8_0 = mtp.tile([128, 2, 2, T], fp8, tag="memt8",
                                     name="memt8_first")
                  nc.gpsimd.dma_start(out=memt8_0[:], in_=memt8_d.ap()[0])
                  shf8_all = []
                  for b in range(PB):
                      shf8 = shfp.tile([16, 2, T], fp8, tag="shf8",
                                       name=f"shf8_{b}")
                      src = bass.AP(tensor=spadb8_d, offset=b * SPADW,
                                    ap=[[2, 16], [1, 2], [1, T]])
                      nc.sync.dma_start(out=shf8[:], in_=src)
                      shf8_all.append(shf8)
                  wqwe_sb = consts.tile([128, 8, 2, 128], fp8, tag="wqwe")
                  nc.gpsimd.dma_start(out=wqwe_sb[:], in_=wqwe_d.ap())
                  onespb = consts.tile([1, PB], bf16, tag="onespb")
                  nc.vector.memset(onespb[:], 1.0)
                  ones128 = consts.tile([1, 128], f32, tag="ones128")
                  nc.vector.memset(ones128[:], 1.0)
                  ones128c = consts.tile([128, 1], f32, tag="ones128c")
                  nc.vector.memset(ones128c[:], 1.0)
              # ---- remaining constants (needed later in the pipeline) ----
              statet_sb = consts.tile([128, PB, NT], bf16, tag="statet")
              nc.sync.dma_start(out=statet_sb[:], in_=statet_d.ap())
              wmb_sb = consts.tile([128, 4, U], bf16, tag="wmb")
              nc.gpsimd.dma_start(out=wmb_sb[:], in_=wmb_d.ap())
              bmrow_sb = consts.tile([1, U], bf16, tag="bmrow")
              nc.sync.dma_start(out=bmrow_sb[:], in_=bmrow_d.ap())
              sigt_sb = consts.tile([1, 2, 128], bf16, tag="sigt")
              nc.sync.dma_start(out=sigt_sb[:], in_=sigt_d.ap())
              idb_sb = consts.tile([128, 128], bf16, tag="idb")
              nc.sync.dma_start(out=idb_sb[:], in_=idb_d.ap())
              call4_sb = [consts.tile([128, ENC], bf16, tag=f"call4_{g}",
                                      name=f"call4_{g}")
                          for g in range(2)]
              for g in range(2):
                  nc.vector.memset(call4_sb[g][:], 0.0)

              # ---- q path: rT[u, vch, b] = (q1 @ WqWe + c0)^T ----
              rT_sb = consts.tile([128, 2, PB], bf16, tag="rT")
              for vch in range(2):
                  rT_ps = psEn.tile([128, PB], f32, tag="enT",
                                    name=f"rTps{vch}")
                  for j in range(8):
                      nc.tensor.matmul(rT_ps[:], wqwe_sb[:, j, vch, :],
                                       q1t_sb[:, j, :],
                                       start=(j == 0), stop=False)
                  nc.tensor.matmul(rT_ps[:], c0row_sb[:, vch, :], onespb[:],
                                   start=False, stop=True)
                  nc.vector.tensor_copy(rT_sb[:, vch, :], rT_ps[:])

              # ---- per-batch pipeline ----
              def load_att(b):
                  if b == 0:
                      return {"memt8": memt8_0, "shf8": shf8_all[0]}
                  memt8 = mtp.tile([128, 2, 2, T], fp8, tag="memt8",
                                   name=f"memt8_{b}")
                  nc.gpsimd.dma_start(out=memt8[:], in_=memt8_d.ap()[b])
                  return {"memt8": memt8, "shf8": shf8_all[b]}

              def load_nat(st, b):
                  natb = natp.tile([128, NT, ENC], bf16, tag="natb",
                                   name=f"natb{b}")
                  nc.gpsimd.dma_start(out=natb[:], in_=natc_d.ap()[b])
                  st["natb"] = natb

              def attention_b(b, st):
                  """PE stream: pre-GEMMs (tb-pairs, [128,1024] 2-bank PSUM
                  tiles) with the previous pair's energy MMs interleaved, so
                  energy never head-of-line blocks on tanh. Ends with the
                  sigmoid. Returns norm state."""
                  memt8, shf8 = st["memt8"], st["shf8"]
                  enT_ps = psEn.tile([128, NT], f32, tag="enT")
                  h8s = []
                  for tp in range(2):
                      h8 = hbp.tile([128, 2, 1024], fp8, tag="h8")
                      h8s.append(h8)
                      for vch in range(2):
                          pre_ps = psPre.tile([128, 1024], f32, tag="pre")
                          for hh in range(2):
                              tb = tp * 2 + hh
                              for cp in range(2):
                                  nc.tensor.matmul(
                                      pre_ps[:, hh * 512:(hh + 1) * 512],
                                      wmwe8_sb[:, cp, :, vch * 128:(vch + 1) * 128],
                                      memt8[:, cp, :, tb * 512:(tb + 1) * 512],
                                      start=(cp == 0), stop=False, perf_mode=DR)
                              nc.tensor.matmul(
                                  pre_ps[:, hh * 512:(hh + 1) * 512],
                                  g8_sb[:, :, vch * 128:(vch + 1) * 128],
                                  shf8[:, :, tb * 512:(tb + 1) * 512],
                                  start=False, stop=True, perf_mode=DR)
                          # fast PSUM->SBUF drain on DVE/GPSIMD (alternating)
                          # so PE's pre tiles free without waiting for tanh
                          pre_sb = prebp.tile([128, 1024], bf16, tag="preb")
                          nc.vector.tensor_copy(pre_sb[:], pre_ps[:])
                          nc.scalar.activation(h8[:, vch, :], pre_sb[:],
                                               AF.Tanh,
                                               bias=rT_sb[:, vch, b:b + 1])
                      if tp == 1:
                          _energy_mms(enT_ps, h8s[0], 0)
                  return {"enT_ps": enT_ps, "h8_tp1": h8s[1], "b": b}

              def finish_att_b(nst):
                  """Second-half energy MMs + sigmoid — issued after the
                  previous batch's context so PE isn't head-of-line blocked
                  waiting for the last tanh."""
                  _energy_mms(nst["enT_ps"], nst["h8_tp1"], 1)
                  sT = rowp.tile([128, NT], bf16, tag="sT")
                  ssum128 = rowp.tile([128, 1], f32, tag="ssum128")
                  nc.scalar.activation(sT[:], nst["enT_ps"][:], AF.Sigmoid,
                                       accum_out=ssum128[:])
                  nst["sT"] = sT
                  nst["ssum128"] = ssum128

              def _energy_mms(enT_ps, h8, tp):
                  for c8 in range(8):
                      nc.tensor.matmul(
                          enT_ps[:, tp * 8 + c8:tp * 8 + c8 + 1],
                          h8[:, :, c8 * 128:(c8 + 1) * 128],
                          vat8_sb[:],
                          start=True, stop=True, perf_mode=DR)

              def norm_b(nst):
                  """Tiny PE/DVE chain: total sigmoid sum -> 1/sum broadcast
                  -> w^T = s^T * rec + state^T."""
                  b = nst["b"]
                  enT_ps = nst["enT_ps"]  # dead after sigmoid; reuse its bank
                  nc.tensor.matmul(enT_ps[0:1, 0:1], ones128c[:],
                                   nst["ssum128"][:],
                                   start=True, stop=True,
                                   skip_group_check=True)
                  rec = rowp.tile([1, 1], f32, tag="rec")
                  nc.vector.reciprocal(rec[:], enT_ps[0:1, 0:1])
                  nc.tensor.matmul(enT_ps[:, 1:2], ones128[:], rec[:],
                                   start=True, stop=True,
                                   skip_group_check=True)
                  wT = rowp.tile([128, NT], bf16, tag="wT")
                  nc.vector.scalar_tensor_tensor(
                      wT[:], in0=nst["sT"][:], scalar=enT_ps[:, 1:2],
                      in1=statet_sb[:, b, :], op0=ALU.mult, op1=ALU.add)
                  nst["wT"] = wT

              def context_b(nst, st):
                  b = nst["b"]
                  natb, wT = st["natb"], nst["wT"]
                  rb = b % 4
                  cv4 = st["cv4"]
                  for ch in range(NT):
                      nc.tensor.matmul(cv4[32 * rb:32 * rb + 1, :],
                                       wT[:, ch:ch + 1], natb[:, ch, :],
                                       start=(ch == 0), stop=(ch == NT - 1),
                                       tile_position=(0, 32 * rb))

              def finish_group(g, cv4, nrows=4, close=True):
                  """Wm contraction for one 4-batch group (first `nrows`
                  rows). With close=False the accumulation group is left open
                  (no stop, no output DMA) for a later direct-path addend."""
                  callT_sb = consts.tile([128, 4, 128], bf16, tag=f"callT{g}",
                                         name=f"callT{g}")
                  # full-partition copy (strided-partition APs are illegal on
                  # DVE); rows not at 32-stride carry PSUM garbage that is
                  # never read. For nrows=3 stop at row 95 so row 96 stays
                  # zero for the last batch's direct-path accumulation.
                  nprt = 128 if nrows == 4 else 32 * nrows
                  nc.vector.tensor_copy(call4_sb[g][0:nprt, :],
                                        cv4[0:nprt, :])
                  # phased bank reuse of cv4 (all call values are in SBUF now):
                  # transposes land in bytes 1024-2047 (bf16 view cols
                  # 512-1023), then the Wm accumulation overwrites f32 cols
                  # 0-255 (bytes 0-1023). Any start=True marks the whole 2KB
                  # region pending, so all transposes strictly precede the
                  # accumulation group.
                  cv4_bf = cv4.bitcast(bf16)
                  for c in range(4):
                      nc.tensor.matmul(cv4_bf[:, 512 + c * 128:512 + (c + 1) * 128],
                                       call4_sb[g][:, c * 128:(c + 1) * 128],
                                       idb_sb[:], is_transpose=True,
                                       skip_group_check=True)
                  nc.vector.tensor_copy(callT_sb[:], cv4_bf[:, 512:1024])
                  ctx_ps = cv4[:, 0:U]
                  for c in range(4):
                      nc.tensor.matmul(ctx_ps, callT_sb[:, c, :],
                                       wmb_sb[:, c, :],
                                       start=(c == 0), stop=False,
                                       skip_group_check=True)
                  nc.tensor.matmul(ctx_ps, sigt_sb[:, g, :], bmrow_sb[:],
                                   start=False, stop=not close,
                                   skip_group_check=True)
                  if close:
                      close_group(g, cv4)

              def close_group(g, cv4):
                  out_sb = consts.tile([128, U], f32, tag=f"outsb{g}",
                                       name=f"outsb{g}")
                  nc.vector.tensor_copy(out_sb[:], cv4[:, 0:U])
                  nc.sync.dma_start(out=out_d.ap()[g * 4:(g + 1) * 4, :],
                                    in_=out_sb[0:128:32, :])

              def last_batch_direct(nst, st, cv4):
                  """Transpose-free tail for the final batch: N=1 GEMVs with
                  the nat chunks stationary produce cv^T [128e, 4] directly
                  (into the dead enT bank), then 4 Wm GEMVs accumulate into
                  the open group's output row 96."""
                  enT_ps = nst["enT_ps"]
                  for ec in range(4):
                      for ch in range(NT):
                          nc.tensor.matmul(
                              enT_ps[:, 4 + ec:5 + ec],
                              st["natb"][:, ch, ec * 128:(ec + 1) * 128],
                              nst["wT"][:, ch:ch + 1],
                              start=(ch == 0), stop=(ch == NT - 1),
                              skip_group_check=True)
                  cvT_sb = rowp.tile([128, 4], bf16, tag="cvT")
                  nc.vector.tensor_copy(cvT_sb[:], enT_ps[:, 4:8])
                  ctx_ps = cv4[:, 0:U]
                  for ec in range(4):
                      nc.tensor.matmul(ctx_ps[96:97, :], cvT_sb[:, ec:ec + 1],
                                       wmb_sb[:, ec, :],
                                       start=False, stop=(ec == 3),
                                       tile_position=(0, 96),
                                       skip_group_check=True)
                  close_group(1, cv4)

              cv4_tiles = {}
              sts = {0: load_att(0)}
              prev = None  # (nst, st) of batch b-1
              for b in range(PB):
                  if b + 1 < PB:
                      sts[b + 1] = load_att(b + 1)
                  st = sts.pop(b)
                  load_nat(st, b)
                  if b % 4 == 0:
                      cv4_tiles[b // 4] = psCv.tile([128, ENC], f32,
                                                    tag="cv4",
                                                    name=f"cv4_{b // 4}")
                  st["cv4"] = cv4_tiles[b // 4]
                  if prev is not None:
                      norm_b(prev[0])
                  nst = attention_b(b, st)
                  if prev is not None:
                      context_b(prev[0], prev[1])
                  finish_att_b(nst)
                  if prev is not None:
                      pb = prev[0]["b"]
                      if pb == 3:
                          finish_group(0, cv4_tiles[0])
                      elif pb == 6:
                          finish_group(1, cv4_tiles[1], nrows=3, close=False)
                  prev = (nst, st)
              norm_b(prev[0])
              last_batch_direct(prev[0], prev[1], cv4_tiles[1])

          for _rep in range(repeat):
              _body()
    nc.compile()
    return nc


def _host_prep(inputs):
    """Fold weights on host (weight-only transforms) and shard per core."""
    f32 = np.float32
    Wq = np.asarray(inputs["Wq"], f32)
    bq = np.asarray(inputs["bq"], f32)
    Wm = np.asarray(inputs["Wm"], f32)
    bm = np.asarray(inputs["bm"], f32)
    Wl = np.asarray(inputs["Wl"], f32)
    bl = np.asarray(inputs["bl"], f32)
    conv_w = np.asarray(inputs["conv_w"], f32)
    conv_b = np.asarray(inputs["conv_b"], f32)
    We = np.asarray(inputs["We"], f32)
    be = np.asarray(inputs["be"], f32)
    v_a = np.asarray(inputs["v_a"], f32)

    WmWe = (Wm @ We).astype(f32)
    WlWe = (Wl @ We).astype(f32)
    WqWe = (Wq @ We).astype(f32)
    G = (conv_w[:, 0, :].T @ WlWe).astype(f32)          # [31, 256]
    c0 = ((bq + bm + bl) @ We + be + conv_b @ WlWe).astype(f32)

    query = np.asarray(inputs["query"], f32)
    state = np.asarray(inputs["state"], f32)
    memory = np.asarray(inputs["memory"], f32)

    # shared (replicated) weight staging
    wmwe8 = np.ascontiguousarray(
        WmWe.reshape(2, 2, 128, U).transpose(2, 0, 1, 3)).astype(FP8)
    G32 = np.zeros((32, U), f32)
    G32[:K] = G
    g8 = np.ascontiguousarray(G32.reshape(16, 2, U)).astype(FP8)
    vat8 = np.ascontiguousarray(v_a.reshape(2, 128).T).reshape(128, 2, 1).astype(FP8)
    wqwe = np.ascontiguousarray(
        WqWe.reshape(8, 128, 2, 128).transpose(1, 0, 2, 3)).astype(FP8)
    c0row = np.ascontiguousarray(c0.reshape(1, 2, 128)).astype(BF16)
    wmb = np.ascontiguousarray(
        Wm.reshape(4, 128, U).transpose(1, 0, 2)).astype(BF16)
    bmrow = bm.reshape(1, U).astype(f32)
    idb = np.eye(128, dtype=f32).astype(BF16)

    q1 = query[:, 1, :]  # [B, HID]
    spadb = np.zeros((B, SPADW), f32)
    spadb[:, PAD:PAD + T] = state
    stsum1 = state.sum(axis=1) + 1.0  # [B]

    shared = {
        "wmwe8": wmwe8, "g8": g8, "vat8": vat8, "wqwe": wqwe,
        "c0row": c0row, "wmb": wmb, "bmrow": bmrow.astype(BF16), "idb": idb,
    }
    in_maps = []
    for c in range(N_CORES):
        sl = slice(c * PB, (c + 1) * PB)
        m = dict(shared)
        mem_c = memory[sl]  # [PB, T, ENC]
        # natc[b, p, i*ENC+e] = mem[b, i*128+p, e]
        m["natc"] = np.ascontiguousarray(
            mem_c.reshape(PB, NT, 128, ENC).transpose(0, 2, 1, 3)
            .reshape(PB, 128, NT * ENC)).astype(BF16)
        # memt8[b, p, cp, i, t] = mem[b, t, (2cp+i)*128+p]
        m["memt8"] = np.ascontiguousarray(
            mem_c.reshape(PB, T, 2, 2, 128).transpose(0, 4, 2, 3, 1)).astype(FP8)
        m["spadb8"] = np.ascontiguousarray(spadb[sl]).astype(FP8)
        # statet[p, b, i] = state[b, i*128+p]
        m["statet"] = np.ascontiguousarray(
            state[sl].reshape(PB, NT, 128).transpose(2, 0, 1)).astype(BF16)
        # q1t[p, j, b] = q1[b, j*128+p]
        m["q1t"] = np.ascontiguousarray(
            q1[sl].reshape(PB, 8, 128).transpose(2, 1, 0)).astype(FP8)
        sigt = np.zeros((1, 2, 128), f32)
        for b in range(PB):
            sigt[0, b // 4, 32 * (b % 4)] = stsum1[c * PB + b]
        m["sigt"] = sigt.astype(BF16)
        in_maps.append(m)
    return in_maps


def kernel(**inputs) -> np.ndarray:
    global LAST_RESULTS
    from concourse import bass_utils

    if "nc" not in _BUILT:
        _BUILT["nc"] = _build_nc()
    nc = _BUILT["nc"]

    in_maps = _host_prep(inputs)
    res = bass_utils.run_bass_kernel_spmd(
        nc, in_maps, core_ids=list(range(N_CORES)), trace=TRACE)
    LAST_RESULTS = res
    out = np.concatenate([res.results[c]["out"] for c in range(N_CORES)], axis=0)
    return out.reshape(B, 1, U).astype(np.float32)
